# revision 6
# baseline (speedup 1.0000x reference)
"""Trainium2 Bass kernel for nn_ConvLayerWithStyleMod.

Math: reference = per-sample style-modulated 3x3 conv_transpose (stride 2)
followed by a fixed separable 4x4 blur ([1,3,3,1] outer [1,3,3,1]).

Decomposition used here (validated vs the reference to ~4e-7):
 - Fold ONLY the vertical blur axis into the conv weights. Splitting the
   dilated output grid by (row parity pr, col parity rc) leaves, per output
   row-parity, two column-parity planes Z0 (even dilated cols, 6 taps) and
   Z1 (odd dilated cols, 3 taps) -- 9 shifted matmuls per row chunk instead
   of the 18 a full 2D fold needs. Both pr phases pack into M=128.
 - The horizontal blur [1,3,3,1] = [1,1]*[1,1]*[1,1] (binomial) is three
   2-tap add stages on the cheap engines, in parity form:
     A0 = Z0[b] + Z1[b]         A1 = Z1[b] + Z0[b+1]
     B0 = A0[b] + A1[b]         B1 = A1[b] + A0[b+1]
     out0[b] = B1[b-1] + B0[b]  out1[b] = B0[b] + B1[b]
   (a global x0.25 for the two blur-axis normalizations is baked into the
   host-folded weights).
 - fp16 for x / weights / blur intermediates (PE rate is identical to f32r,
   DMA-in halves, DVE adds get the 2x packed mode); PSUM accum + final
   output stay f32.

Sharding: data-parallel over batch; sample i on core i. Output is written
as [p=(pr,o), a, c] (128 partitions) so each chunk is ONE full-width DMA;
host de-interleaves rows at the end.
"""

import math

import numpy as np

B, C, OC, SD, H = 8, 128, 64, 512, 128
KW = 3
EPS = 1e-8
HP = H + 2          # zero-padded image size in SBUF (rows/cols -1..128)
NCORES = 8
RB = 3              # image rows per chunk (PSUM bank: 3*130 = 390 <= 512 f32)
NTAP = 9            # 6 Z0 taps + 3 Z1 taps

_prog = None


def _host_z_kernels(style, weight, mod_weight, mod_bias):
    """Per-sample vertically-blur-folded kernels as matmul lhsT, f16.

    Returns (B, C, 9*128) float16 where tap t column block t*128 + pr*64 + o:
      t = d*2 + e (d row tap 0..2, e col shift 0..1)  -> Z0 taps
      t = 6 + d                                        -> Z1 taps
    """
    style = np.asarray(style, dtype=np.float64)
    weight = np.asarray(weight, dtype=np.float64)
    mod_weight = np.asarray(mod_weight, dtype=np.float64)
    mod_bias = np.asarray(mod_bias, dtype=np.float64)

    b = style.shape[0]
    scale = 1.0 / math.sqrt(C * KW * KW)
    w_gain = 1.0 / math.sqrt(mod_weight.shape[1])
    s = style @ (mod_weight.T * w_gain) + mod_bias             # (b, C)
    wmod = scale * weight * s[:, None, :, None, None]          # (b, OC, C, 3, 3)
    demod = 1.0 / np.sqrt(np.sum(wmod * wmod, axis=(2, 3, 4)) + EPS)
    wt = wmod * demod[:, :, None, None, None]                  # conv_transpose taps

    # vertical blur fold: out dilated row J = 2a+pr reads x row a+dlt with
    # FIR tap u = 2*dlt + dr + 1 - pr ; f1 = [1,3,3,1]/4, extra 0.25 for the
    # unscaled horizontal binomial stages.
    f1 = np.array([1.0, 3.0, 3.0, 1.0]) / 4.0
    V = np.zeros((2, 3, 3))
    for pr in range(2):
        for dlt in (-1, 0, 1):
            for dr in range(3):
                u = 2 * dlt + dr + 1 - pr
                if 0 <= u <= 3:
                    V[pr, dlt + 1, dr] += f1[u] * 0.25

    wts = np.zeros((b, C, NTAP * 128), dtype=np.float64)
    # wt[b, o, c, dr, dc] -> per tap block [c, pr*64+o]
    wtT = wt.transpose(0, 2, 1, 3, 4)                          # (b, C, OC, 3, 3)
    for pr in range(2):
        for d in range(3):
            # Z0: dc = 2e (e = col shift); Z1: dc = 1
            acc0 = np.zeros((b, C, OC, 2))
            acc1 = np.zeros((b, C, OC))
            for dr in range(3):
                v = V[pr, d, dr]
                if v == 0.0:
                    continue
                acc0[..., 0] += v * wtT[:, :, :, dr, 0]
                acc0[..., 1] += v * wtT[:, :, :, dr, 2]
                acc1 += v * wtT[:, :, :, dr, 1]
            for e in range(2):
                t = d * 2 + e
                wts[:, :, t * 128 + pr * 64:t * 128 + pr * 64 + OC] = acc0[..., e]
            t = 6 + d
            wts[:, :, t * 128 + pr * 64:t * 128 + pr * 64 + OC] = acc1
    return wts.astype(np.float16)


def _build():
    import concourse.bacc as bacc
    import concourse.mybir as mybir
    from concourse.tile import TileContext

    f32 = mybir.dt.float32
    f16 = mybir.dt.float16
    AOp = mybir.AluOpType

    nc = bacc.Bacc(None, target_bir_lowering=False)
    x = nc.declare_dram_parameter("x", [C, HP * HP], f16, isOutput=False)
    wts = nc.declare_dram_parameter("wts", [C, NTAP * 128], f16, isOutput=False)
    # out[p = pr*64 + o, a, c]; host interleaves rows (2a+pr) afterwards
    out = nc.declare_dram_parameter("out", [128, H, 2 * H], f32, isOutput=True)

    with TileContext(nc) as tc:
        with (
            tc.tile_pool(name="xp", bufs=1) as xpool,
            tc.tile_pool(name="wp", bufs=1) as wpool,
            tc.tile_pool(name="ps", bufs=4, space="PSUM") as pspool,
            tc.tile_pool(name="zc", bufs=5) as zcpool,
            tc.tile_pool(name="ab", bufs=5) as abpool,
            tc.tile_pool(name="ob", bufs=8) as opool,
        ):
            xpad = xpool.tile([C, HP * HP], f16)
            xv = xpad[:, :].rearrange("p (r c) -> p r c", c=HP)
            wtile = wpool.tile([C, NTAP * 128], f16)

            # x arrives pre-padded; band loads so compute starts early.
            # Small first band gates only the first chunk; weights ride
            # second so the PE can start ~2.5us in.
            xdram = x.rearrange("p (r c) -> p r c", c=HP)
            bands = [(0, 6)]
            r0 = 6
            while r0 < HP:
                r1 = min(HP, r0 + 12)
                bands.append((r0, r1))
                r0 = r1
            nc.sync.dma_start(out=xv[:, 0:6, :], in_=xdram[:, 0:6, :])
            nc.sync.dma_start(out=wtile[:, :], in_=wts[:, :])
            for r0, r1 in bands[1:]:
                nc.sync.dma_start(out=xv[:, r0:r1, :], in_=xdram[:, r0:r1, :])

            nchunks = (H + RB - 1) // RB
            for ci in range(nchunks):
                a0 = ci * RB
                rb = min(RB, H - a0)
                z0 = pspool.tile([C, RB, H + 1], f32, tag="z0", name=f"z0_{ci}")
                z1 = pspool.tile([C, RB, H + 2], f32, tag="z1", name=f"z1_{ci}")
                # Z0: 6 taps (d rows x e col-shifts); x col idx b-e+1
                for d in range(3):
                    for e in range(2):
                        t = d * 2 + e
                        nc.tensor.matmul(
                            z0[:, 0:rb, :],
                            wtile[:, t * 128:(t + 1) * 128],
                            xv[:, a0 + d:a0 + d + rb, 1 - e:HP - e - 1 + 1],
                            start=(t == 0),
                            stop=(t == 5),
                        )
                # Z1: 3 taps; full padded col range
                for d in range(3):
                    t = 6 + d
                    nc.tensor.matmul(
                        z1[:, 0:rb, :],
                        wtile[:, t * 128:(t + 1) * 128],
                        xv[:, a0 + d:a0 + d + rb, 0:HP],
                        start=(d == 0),
                        stop=(d == 2),
                    )
                # PSUM -> SBUF f16 casts on the (otherwise idle) ACT engine
                z0c = zcpool.tile([C, RB, H + 1], f16, tag="z0c", name=f"z0c_{ci}")
                z1c = zcpool.tile([C, RB, H + 2], f16, tag="z1c", name=f"z1c_{ci}")
                nc.scalar.copy(z0c[:, 0:rb, :], z0[:, 0:rb, :])
                nc.scalar.copy(z1c[:, 0:rb, :], z1[:, 0:rb, :])
                # binomial blur stages (DVE, f16 packed 2x)
                A0 = abpool.tile([C, RB, H + 1], f16, tag="A0", name=f"A0_{ci}")
                A1 = abpool.tile([C, RB, H + 1], f16, tag="A1", name=f"A1_{ci}")
                B0 = abpool.tile([C, RB, H], f16, tag="B0", name=f"B0_{ci}")
                B1 = abpool.tile([C, RB, H + 1], f16, tag="B1", name=f"B1_{ci}")
                nc.vector.tensor_tensor(
                    out=A0[:, 0:rb, :], in0=z0c[:, 0:rb, :],
                    in1=z1c[:, 0:rb, 1:H + 2], op=AOp.add)
                nc.vector.tensor_tensor(
                    out=A1[:, 0:rb, :], in0=z1c[:, 0:rb, 0:H + 1],
                    in1=z0c[:, 0:rb, :], op=AOp.add)
                nc.vector.tensor_tensor(
                    out=B0[:, 0:rb, :], in0=A0[:, 0:rb, 0:H],
                    in1=A1[:, 0:rb, 1:H + 1], op=AOp.add)
                nc.vector.tensor_tensor(
                    out=B1[:, 0:rb, :], in0=A1[:, 0:rb, :],
                    in1=A0[:, 0:rb, :], op=AOp.add)
                # final interleaved f32 writes; Pool can't run TensorScalarPtr
                # so these are plain adds, load-balanced DVE/Pool
                osb = opool.tile([C, RB, 2 * H], f32, tag="osb", name=f"osb_{ci}")
                eng0 = nc.vector if ci % 2 == 0 else nc.gpsimd
                eng0.tensor_tensor(
                    out=osb[:, 0:rb, 0::2], in0=B1[:, 0:rb, 0:H],
                    in1=B0[:, 0:rb, :], op=AOp.add)
                nc.gpsimd.tensor_tensor(
                    out=osb[:, 0:rb, 1::2], in0=B0[:, 0:rb, :],
                    in1=B1[:, 0:rb, 1:H + 1], op=AOp.add)
                nc.sync.dma_start(
                    out=out[:, a0:a0 + rb, :], in_=osb[:, 0:rb, :])
    nc.compile()
    return nc


def _get_prog():
    global _prog
    if _prog is None:
        _prog = _build()
    return _prog


def _pad_x(xi):
    xp = np.zeros((C, HP, HP), dtype=np.float16)
    xp[:, 1:1 + H, 1:1 + H] = xi
    return xp.reshape(C, HP * HP)


def kernel(x, style, weight, mod_weight, mod_bias):
    from concourse.bass_utils import run_bass_kernel_spmd

    nc = _get_prog()
    wts = _host_z_kernels(style, weight, mod_weight, mod_bias)
    x = np.asarray(x)
    in_maps = [
        {"x": _pad_x(x[i]), "wts": np.ascontiguousarray(wts[i])}
        for i in range(NCORES)
    ]
    r = run_bass_kernel_spmd(nc, in_maps, list(range(NCORES)))
    outs = []
    for i in range(NCORES):
        o = r.results[i]["out"]                    # (128, 128, 256) [pr*64+o, a, c]
        o = o.reshape(2, OC, H, 2 * H).transpose(1, 2, 0, 3)
        outs.append(o.reshape(OC, 2 * H, 2 * H))
    return np.stack(outs, axis=0).astype(np.float32)


# revision 9
# speedup vs baseline: 1.0424x; 1.0424x over previous
"""Trainium2 Bass kernel for nn_ConvLayerWithStyleMod.

Math: reference = per-sample style-modulated 3x3 conv_transpose (stride 2)
followed by a fixed separable 4x4 blur ([1,3,3,1] outer [1,3,3,1]).

Decomposition used here (validated vs the reference to ~4e-7):
 - Fold ONLY the vertical blur axis into the conv weights. Splitting the
   dilated output grid by (row parity pr, col parity rc) leaves, per output
   row-parity, two column-parity planes Z0 (even dilated cols, 6 taps) and
   Z1 (odd dilated cols, 3 taps) -- 9 shifted matmuls per row chunk instead
   of the 18 a full 2D fold needs. Both pr phases pack into M=128.
 - The horizontal blur [1,3,3,1] = [1,1]*[1,1]*[1,1] (binomial) is three
   2-tap add stages on the cheap engines, in parity form:
     A0 = Z0[b] + Z1[b]         A1 = Z1[b] + Z0[b+1]
     B0 = A0[b] + A1[b]         B1 = A1[b] + A0[b+1]
     out0[b] = B1[b-1] + B0[b]  out1[b] = B0[b] + B1[b]
   (a global x0.25 for the two blur-axis normalizations is baked into the
   host-folded weights).
 - fp16 for x / weights / blur intermediates (PE rate is identical to f32r,
   DMA-in halves, DVE adds get the 2x packed mode); PSUM accum + final
   output stay f32.

Sharding: data-parallel over batch; sample i on core i. Output is written
as [p=(pr,o), a, c] (128 partitions) so each chunk is ONE full-width DMA;
host de-interleaves rows at the end.
"""

import math

import numpy as np

B, C, OC, SD, H = 8, 128, 64, 512, 128
KW = 3
EPS = 1e-8
HP = H + 2          # zero-padded image size in SBUF (rows/cols -1..128)
NCORES = 8
RB = 3              # image rows per chunk (PSUM bank: 3*130 = 390 <= 512 f32)
NTAP = 9            # 6 Z0 taps + 3 Z1 taps

_prog = None


def _host_z_kernels(style, weight, mod_weight, mod_bias):
    """Per-sample vertically-blur-folded kernels as matmul lhsT, f16.

    Returns (B, C, 9*128) float16 where tap t column block t*128 + pr*64 + o:
      t = d*2 + e (d row tap 0..2, e col shift 0..1)  -> Z0 taps
      t = 6 + d                                        -> Z1 taps
    """
    style = np.asarray(style, dtype=np.float64)
    weight = np.asarray(weight, dtype=np.float64)
    mod_weight = np.asarray(mod_weight, dtype=np.float64)
    mod_bias = np.asarray(mod_bias, dtype=np.float64)

    b = style.shape[0]
    scale = 1.0 / math.sqrt(C * KW * KW)
    w_gain = 1.0 / math.sqrt(mod_weight.shape[1])
    s = style @ (mod_weight.T * w_gain) + mod_bias             # (b, C)
    wmod = scale * weight * s[:, None, :, None, None]          # (b, OC, C, 3, 3)
    demod = 1.0 / np.sqrt(np.sum(wmod * wmod, axis=(2, 3, 4)) + EPS)
    wt = wmod * demod[:, :, None, None, None]                  # conv_transpose taps

    # vertical blur fold: out dilated row J = 2a+pr reads x row a+dlt with
    # FIR tap u = 2*dlt + dr + 1 - pr ; f1 = [1,3,3,1]/4, extra 0.25 for the
    # unscaled horizontal binomial stages.
    f1 = np.array([1.0, 3.0, 3.0, 1.0]) / 4.0
    V = np.zeros((2, 3, 3))
    for pr in range(2):
        for dlt in (-1, 0, 1):
            for dr in range(3):
                u = 2 * dlt + dr + 1 - pr
                if 0 <= u <= 3:
                    V[pr, dlt + 1, dr] += f1[u] * 0.25

    wts = np.zeros((b, C, NTAP * 128), dtype=np.float64)
    # wt[b, o, c, dr, dc] -> per tap block [c, pr*64+o]
    wtT = wt.transpose(0, 2, 1, 3, 4)                          # (b, C, OC, 3, 3)
    for pr in range(2):
        for d in range(3):
            # Z0: dc = 2e (e = col shift); Z1: dc = 1
            acc0 = np.zeros((b, C, OC, 2))
            acc1 = np.zeros((b, C, OC))
            for dr in range(3):
                v = V[pr, d, dr]
                if v == 0.0:
                    continue
                acc0[..., 0] += v * wtT[:, :, :, dr, 0]
                acc0[..., 1] += v * wtT[:, :, :, dr, 2]
                acc1 += v * wtT[:, :, :, dr, 1]
            for e in range(2):
                t = d * 2 + e
                wts[:, :, t * 128 + pr * 64:t * 128 + pr * 64 + OC] = acc0[..., e]
            t = 6 + d
            wts[:, :, t * 128 + pr * 64:t * 128 + pr * 64 + OC] = acc1
    return wts.astype(np.float16)


def _build():
    import concourse.bacc as bacc
    import concourse.mybir as mybir
    from concourse.tile import TileContext

    f32 = mybir.dt.float32
    f16 = mybir.dt.float16
    AOp = mybir.AluOpType

    nc = bacc.Bacc(None, target_bir_lowering=False)
    x = nc.declare_dram_parameter("x", [C, HP * HP], f16, isOutput=False)
    wts = nc.declare_dram_parameter("wts", [C, NTAP * 128], f16, isOutput=False)
    # out[p = pr*64 + o, a, c]; host interleaves rows (2a+pr) afterwards
    out = nc.declare_dram_parameter("out", [128, H, 2 * H], f32, isOutput=True)

    with TileContext(nc) as tc:
        with (
            tc.tile_pool(name="xp", bufs=1) as xpool,
            tc.tile_pool(name="wp", bufs=1) as wpool,
            tc.tile_pool(name="ps", bufs=4, space="PSUM") as pspool,
            tc.tile_pool(name="zc", bufs=5) as zcpool,
            tc.tile_pool(name="ab", bufs=5) as abpool,
            tc.tile_pool(name="ob", bufs=8) as opool,
        ):
            xpad = xpool.tile([C, HP * HP], f16)
            xv = xpad[:, :].rearrange("p (r c) -> p r c", c=HP)
            wtile = wpool.tile([C, NTAP * 128], f16)

            # x arrives pre-padded; band loads so compute starts early.
            # Small first band gates only the first chunk; weights ride
            # second so the PE can start ~2.5us in.
            xdram = x.rearrange("p (r c) -> p r c", c=HP)
            nc.sync.dma_start(out=wtile[:, :], in_=wts[:, :])
            bands = [(0, 6)]
            r0 = 6
            while r0 < HP:
                r1 = min(HP, r0 + 10)
                bands.append((r0, r1))
                r0 = r1
            for r0, r1 in bands:
                nc.sync.dma_start(out=xv[:, r0:r1, :], in_=xdram[:, r0:r1, :])

            nchunks = (H + RB - 1) // RB
            for ci in range(nchunks):
                a0 = ci * RB
                rb = min(RB, H - a0)
                z0 = pspool.tile([C, RB, H + 1], f32, tag="z0", name=f"z0_{ci}")
                z1 = pspool.tile([C, RB, H + 2], f32, tag="z1", name=f"z1_{ci}")
                z0c = zcpool.tile([C, RB, H + 1], f16, tag="z0c", name=f"z0c_{ci}")
                z1c = zcpool.tile([C, RB, H + 2], f16, tag="z1c", name=f"z1c_{ci}")
                # Z1 first: its ACT cast overlaps the 6 Z0 matmuls, shortening
                # the end-of-chunk dependency chain
                for d in range(3):
                    t = 6 + d
                    nc.tensor.matmul(
                        z1[:, 0:rb, :],
                        wtile[:, t * 128:(t + 1) * 128],
                        xv[:, a0 + d:a0 + d + rb, 0:HP],
                        start=(d == 0),
                        stop=(d == 2),
                    )
                nc.scalar.copy(z1c[:, 0:rb, :], z1[:, 0:rb, :])
                # Z0: 6 taps (d rows x e col-shifts); x col idx b-e+1
                for d in range(3):
                    for e in range(2):
                        t = d * 2 + e
                        nc.tensor.matmul(
                            z0[:, 0:rb, :],
                            wtile[:, t * 128:(t + 1) * 128],
                            xv[:, a0 + d:a0 + d + rb, 1 - e:HP - e - 1 + 1],
                            start=(t == 0),
                            stop=(t == 5),
                        )
                nc.scalar.copy(z0c[:, 0:rb, :], z0[:, 0:rb, :])
                # binomial blur stages (DVE, f16 packed 2x)
                A0 = abpool.tile([C, RB, H + 1], f16, tag="A0", name=f"A0_{ci}")
                A1 = abpool.tile([C, RB, H + 1], f16, tag="A1", name=f"A1_{ci}")
                B0 = abpool.tile([C, RB, H], f16, tag="B0", name=f"B0_{ci}")
                B1 = abpool.tile([C, RB, H + 1], f16, tag="B1", name=f"B1_{ci}")
                nc.vector.tensor_tensor(
                    out=A0[:, 0:rb, :], in0=z0c[:, 0:rb, :],
                    in1=z1c[:, 0:rb, 1:H + 2], op=AOp.add)
                nc.vector.tensor_tensor(
                    out=A1[:, 0:rb, :], in0=z1c[:, 0:rb, 0:H + 1],
                    in1=z0c[:, 0:rb, :], op=AOp.add)
                nc.vector.tensor_tensor(
                    out=B0[:, 0:rb, :], in0=A0[:, 0:rb, 0:H],
                    in1=A1[:, 0:rb, 1:H + 1], op=AOp.add)
                nc.vector.tensor_tensor(
                    out=B1[:, 0:rb, :], in0=A1[:, 0:rb, :],
                    in1=A0[:, 0:rb, :], op=AOp.add)
                # final interleaved f32 writes; Pool can't run TensorScalarPtr
                # so these are plain adds, load-balanced DVE/Pool
                osb = opool.tile([C, RB, 2 * H], f32, tag="osb", name=f"osb_{ci}")
                nc.vector.tensor_tensor(
                    out=osb[:, 0:rb, 0::2], in0=B1[:, 0:rb, 0:H],
                    in1=B0[:, 0:rb, :], op=AOp.add)
                nc.gpsimd.tensor_tensor(
                    out=osb[:, 0:rb, 1::2], in0=B0[:, 0:rb, :],
                    in1=B1[:, 0:rb, 1:H + 1], op=AOp.add)
                nc.sync.dma_start(
                    out=out[:, a0:a0 + rb, :], in_=osb[:, 0:rb, :])
    nc.compile()
    return nc


def _get_prog():
    global _prog
    if _prog is None:
        _prog = _build()
    return _prog


def _pad_x(xi):
    xp = np.zeros((C, HP, HP), dtype=np.float16)
    xp[:, 1:1 + H, 1:1 + H] = xi
    return xp.reshape(C, HP * HP)


def kernel(x, style, weight, mod_weight, mod_bias):
    from concourse.bass_utils import run_bass_kernel_spmd

    nc = _get_prog()
    wts = _host_z_kernels(style, weight, mod_weight, mod_bias)
    x = np.asarray(x)
    in_maps = [
        {"x": _pad_x(x[i]), "wts": np.ascontiguousarray(wts[i])}
        for i in range(NCORES)
    ]
    r = run_bass_kernel_spmd(nc, in_maps, list(range(NCORES)))
    outs = []
    for i in range(NCORES):
        o = r.results[i]["out"]                    # (128, 128, 256) [pr*64+o, a, c]
        o = o.reshape(2, OC, H, 2 * H).transpose(1, 2, 0, 3)
        outs.append(o.reshape(OC, 2 * H, 2 * H))
    return np.stack(outs, axis=0).astype(np.float32)


# revision 12
# speedup vs baseline: 1.0460x; 1.0034x over previous
"""Trainium2 Bass kernel for nn_ConvLayerWithStyleMod.

Math: reference = per-sample style-modulated 3x3 conv_transpose (stride 2)
followed by a fixed separable 4x4 blur ([1,3,3,1] outer [1,3,3,1]).

Decomposition used here (validated vs the reference to ~4e-7):
 - Fold ONLY the vertical blur axis into the conv weights. Splitting the
   dilated output grid by (row parity pr, col parity rc) leaves, per output
   row-parity, two column-parity planes Z0 (even dilated cols, 6 taps) and
   Z1 (odd dilated cols, 3 taps) -- 9 shifted matmuls per row chunk instead
   of the 18 a full 2D fold needs. Both pr phases pack into M=128.
 - The horizontal blur [1,3,3,1] = [1,1]*[1,1]*[1,1] (binomial) is three
   2-tap add stages on the cheap engines, in parity form:
     A0 = Z0[b] + Z1[b]         A1 = Z1[b] + Z0[b+1]
     B0 = A0[b] + A1[b]         B1 = A1[b] + A0[b+1]
     out0[b] = B1[b-1] + B0[b]  out1[b] = B0[b] + B1[b]
   (a global x0.25 for the two blur-axis normalizations is baked into the
   host-folded weights).
 - fp16 for x / weights / blur intermediates (PE rate is identical to f32r,
   DMA-in halves, DVE adds get the 2x packed mode); PSUM accum + final
   output stay f32.

Sharding: data-parallel over batch; sample i on core i. Output is written
as [p=(pr,o), a, c] (128 partitions) so each chunk is ONE full-width DMA;
host de-interleaves rows at the end.
"""

import math

import numpy as np

B, C, OC, SD, H = 8, 128, 64, 512, 128
KW = 3
EPS = 1e-8
HP = H + 2          # zero-padded image size in SBUF (rows/cols -1..128)
NCORES = 8
RB = 3              # image rows per chunk (PSUM bank: 3*130 = 390 <= 512 f32)
NTAP = 9            # 6 Z0 taps + 3 Z1 taps

_prog = None


def _host_z_kernels(style, weight, mod_weight, mod_bias):
    """Per-sample vertically-blur-folded kernels as matmul lhsT, f16.

    Returns (B, C, 9*128) float16 where tap t column block t*128 + pr*64 + o:
      t = d*2 + e (d row tap 0..2, e col shift 0..1)  -> Z0 taps
      t = 6 + d                                        -> Z1 taps
    """
    style = np.asarray(style, dtype=np.float64)
    weight = np.asarray(weight, dtype=np.float64)
    mod_weight = np.asarray(mod_weight, dtype=np.float64)
    mod_bias = np.asarray(mod_bias, dtype=np.float64)

    b = style.shape[0]
    scale = 1.0 / math.sqrt(C * KW * KW)
    w_gain = 1.0 / math.sqrt(mod_weight.shape[1])
    s = style @ (mod_weight.T * w_gain) + mod_bias             # (b, C)
    wmod = scale * weight * s[:, None, :, None, None]          # (b, OC, C, 3, 3)
    demod = 1.0 / np.sqrt(np.sum(wmod * wmod, axis=(2, 3, 4)) + EPS)
    wt = wmod * demod[:, :, None, None, None]                  # conv_transpose taps

    # vertical blur fold: out dilated row J = 2a+pr reads x row a+dlt with
    # FIR tap u = 2*dlt + dr + 1 - pr ; f1 = [1,3,3,1]/4, extra 0.25 for the
    # unscaled horizontal binomial stages.
    f1 = np.array([1.0, 3.0, 3.0, 1.0]) / 4.0
    V = np.zeros((2, 3, 3))
    for pr in range(2):
        for dlt in (-1, 0, 1):
            for dr in range(3):
                u = 2 * dlt + dr + 1 - pr
                if 0 <= u <= 3:
                    V[pr, dlt + 1, dr] += f1[u] * 0.25

    wts = np.zeros((b, C, NTAP * 128), dtype=np.float64)
    # wt[b, o, c, dr, dc] -> per tap block [c, pr*64+o]
    wtT = wt.transpose(0, 2, 1, 3, 4)                          # (b, C, OC, 3, 3)
    for pr in range(2):
        for d in range(3):
            # Z0: dc = 2e (e = col shift); Z1: dc = 1
            acc0 = np.zeros((b, C, OC, 2))
            acc1 = np.zeros((b, C, OC))
            for dr in range(3):
                v = V[pr, d, dr]
                if v == 0.0:
                    continue
                acc0[..., 0] += v * wtT[:, :, :, dr, 0]
                acc0[..., 1] += v * wtT[:, :, :, dr, 2]
                acc1 += v * wtT[:, :, :, dr, 1]
            for e in range(2):
                t = d * 2 + e
                wts[:, :, t * 128 + pr * 64:t * 128 + pr * 64 + OC] = acc0[..., e]
            t = 6 + d
            wts[:, :, t * 128 + pr * 64:t * 128 + pr * 64 + OC] = acc1
    return wts.astype(np.float16)


def _build():
    import concourse.bacc as bacc
    import concourse.mybir as mybir
    from concourse.tile import TileContext

    f32 = mybir.dt.float32
    f16 = mybir.dt.float16
    AOp = mybir.AluOpType

    nc = bacc.Bacc(None, target_bir_lowering=False)
    x = nc.declare_dram_parameter("x", [C, HP * HP], f16, isOutput=False)
    wts = nc.declare_dram_parameter("wts", [C, NTAP * 128], f16, isOutput=False)
    # out[p = pr*64 + o, a, c]; host interleaves rows (2a+pr) afterwards
    out = nc.declare_dram_parameter("out", [128, H, 2 * H], f32, isOutput=True)

    with TileContext(nc) as tc:
        with (
            tc.tile_pool(name="xp", bufs=1) as xpool,
            tc.tile_pool(name="wp", bufs=1) as wpool,
            tc.tile_pool(name="ps", bufs=3, space="PSUM") as pspool,
            tc.tile_pool(name="pw", bufs=1, space="PSUM") as pwpool,
            tc.tile_pool(name="zc", bufs=5) as zcpool,
            tc.tile_pool(name="ab", bufs=5) as abpool,
            tc.tile_pool(name="ob", bufs=8) as opool,
        ):
            xpad = xpool.tile([C, HP * HP], f16)
            xv = xpad[:, :].rearrange("p (r c) -> p r c", c=HP)
            wtile = wpool.tile([C, NTAP * 128], f16)

            # x arrives pre-padded; band loads so compute starts early.
            # Small first band gates only the first chunk; weights ride
            # second so the PE can start ~2.5us in.
            xdram = x.rearrange("p (r c) -> p r c", c=HP)
            nc.sync.dma_start(out=wtile[:, :], in_=wts[:, :])
            bands = [(0, 6)]
            r0 = 6
            while r0 < HP:
                r1 = min(HP, r0 + 10)
                bands.append((r0, r1))
                r0 = r1
            for r0, r1 in bands:
                nc.sync.dma_start(out=xv[:, r0:r1, :], in_=xdram[:, r0:r1, :])

            # PE p-state warmup: dummy matmuls on a zeroed tile keep the PE
            # continuously busy from ~0.5us so the ramp (3us to full clock)
            # completes before the first real chunk issues.
            warm = xpool.tile([C, 256], f16, name="warm")
            nc.gpsimd.memset(warm[:, :], 0.0)
            wps = pwpool.tile([C, 256], f32, name="warmps")
            for wi in range(34):
                nc.tensor.matmul(
                    wps[:, :], warm[:, 0:128], warm[:, :],
                    start=True, stop=True)

            sizes = [RB] * ((H - 2) // RB) + [1, 1]
            assert sum(sizes) == H
            a0 = 0
            for ci, rb in enumerate(sizes):
                z0 = pspool.tile([C, RB, H + 1], f32, tag="z0", name=f"z0_{ci}")
                z1 = pspool.tile([C, RB, H + 2], f32, tag="z1", name=f"z1_{ci}")
                z0c = zcpool.tile([C, RB, H + 1], f16, tag="z0c", name=f"z0c_{ci}")
                z1c = zcpool.tile([C, RB, H + 2], f16, tag="z1c", name=f"z1c_{ci}")
                # Z1 first: its ACT cast overlaps the 6 Z0 matmuls, shortening
                # the end-of-chunk dependency chain
                for d in range(3):
                    t = 6 + d
                    nc.tensor.matmul(
                        z1[:, 0:rb, :],
                        wtile[:, t * 128:(t + 1) * 128],
                        xv[:, a0 + d:a0 + d + rb, 0:HP],
                        start=(d == 0),
                        stop=(d == 2),
                    )
                nc.scalar.copy(z1c[:, 0:rb, :], z1[:, 0:rb, :])
                # Z0: 6 taps (d rows x e col-shifts); x col idx b-e+1
                for d in range(3):
                    for e in range(2):
                        t = d * 2 + e
                        nc.tensor.matmul(
                            z0[:, 0:rb, :],
                            wtile[:, t * 128:(t + 1) * 128],
                            xv[:, a0 + d:a0 + d + rb, 1 - e:HP - e - 1 + 1],
                            start=(t == 0),
                            stop=(t == 5),
                        )
                nc.scalar.copy(z0c[:, 0:rb, :], z0[:, 0:rb, :])
                # binomial blur stages (DVE, f16 packed 2x)
                A0 = abpool.tile([C, RB, H + 1], f16, tag="A0", name=f"A0_{ci}")
                A1 = abpool.tile([C, RB, H + 1], f16, tag="A1", name=f"A1_{ci}")
                B0 = abpool.tile([C, RB, H], f16, tag="B0", name=f"B0_{ci}")
                B1 = abpool.tile([C, RB, H + 1], f16, tag="B1", name=f"B1_{ci}")
                nc.vector.tensor_tensor(
                    out=A0[:, 0:rb, :], in0=z0c[:, 0:rb, :],
                    in1=z1c[:, 0:rb, 1:H + 2], op=AOp.add)
                nc.vector.tensor_tensor(
                    out=A1[:, 0:rb, :], in0=z1c[:, 0:rb, 0:H + 1],
                    in1=z0c[:, 0:rb, :], op=AOp.add)
                nc.vector.tensor_tensor(
                    out=B0[:, 0:rb, :], in0=A0[:, 0:rb, 0:H],
                    in1=A1[:, 0:rb, 1:H + 1], op=AOp.add)
                nc.vector.tensor_tensor(
                    out=B1[:, 0:rb, :], in0=A1[:, 0:rb, :],
                    in1=A0[:, 0:rb, :], op=AOp.add)
                # final interleaved f32 writes; Pool can't run TensorScalarPtr
                # so these are plain adds, load-balanced DVE/Pool
                osb = opool.tile([C, RB, 2 * H], f32, tag="osb", name=f"osb_{ci}")
                eng0 = nc.vector if ci % 2 == 0 else nc.gpsimd
                eng0.tensor_tensor(
                    out=osb[:, 0:rb, 0::2], in0=B1[:, 0:rb, 0:H],
                    in1=B0[:, 0:rb, :], op=AOp.add)
                nc.gpsimd.tensor_tensor(
                    out=osb[:, 0:rb, 1::2], in0=B0[:, 0:rb, :],
                    in1=B1[:, 0:rb, 1:H + 1], op=AOp.add)
                nc.sync.dma_start(
                    out=out[:, a0:a0 + rb, :], in_=osb[:, 0:rb, :])
                a0 += rb
    nc.compile()
    return nc


def _get_prog():
    global _prog
    if _prog is None:
        _prog = _build()
    return _prog


def _pad_x(xi):
    xp = np.zeros((C, HP, HP), dtype=np.float16)
    xp[:, 1:1 + H, 1:1 + H] = xi
    return xp.reshape(C, HP * HP)


def kernel(x, style, weight, mod_weight, mod_bias):
    from concourse.bass_utils import run_bass_kernel_spmd

    nc = _get_prog()
    wts = _host_z_kernels(style, weight, mod_weight, mod_bias)
    x = np.asarray(x)
    in_maps = [
        {"x": _pad_x(x[i]), "wts": np.ascontiguousarray(wts[i])}
        for i in range(NCORES)
    ]
    r = run_bass_kernel_spmd(nc, in_maps, list(range(NCORES)))
    outs = []
    for i in range(NCORES):
        o = r.results[i]["out"]                    # (128, 128, 256) [pr*64+o, a, c]
        o = o.reshape(2, OC, H, 2 * H).transpose(1, 2, 0, 3)
        outs.append(o.reshape(OC, 2 * H, 2 * H))
    return np.stack(outs, axis=0).astype(np.float32)


# revision 13
# speedup vs baseline: 1.0875x; 1.0397x over previous
"""Trainium2 Bass kernel for nn_ConvLayerWithStyleMod.

Math: reference = per-sample style-modulated 3x3 conv_transpose (stride 2)
followed by a fixed separable 4x4 blur ([1,3,3,1] outer [1,3,3,1]).

Decomposition used here (validated vs the reference to ~4e-7):
 - Fold ONLY the vertical blur axis into the conv weights. Splitting the
   dilated output grid by (row parity pr, col parity rc) leaves, per output
   row-parity, two column-parity planes Z0 (even dilated cols, 6 taps) and
   Z1 (odd dilated cols, 3 taps) -- 9 shifted matmuls per row chunk instead
   of the 18 a full 2D fold needs. Both pr phases pack into M=128.
 - The horizontal blur [1,3,3,1] = [1,1]*[1,1]*[1,1] (binomial) is three
   2-tap add stages on the cheap engines, in parity form:
     A0 = Z0[b] + Z1[b]         A1 = Z1[b] + Z0[b+1]
     B0 = A0[b] + A1[b]         B1 = A1[b] + A0[b+1]
     out0[b] = B1[b-1] + B0[b]  out1[b] = B0[b] + B1[b]
   (a global x0.25 for the two blur-axis normalizations is baked into the
   host-folded weights).
 - fp16 for x / weights / blur intermediates (PE rate is identical to f32r,
   DMA-in halves, DVE adds get the 2x packed mode); PSUM accum + final
   output stay f32.

Sharding: data-parallel over batch; sample i on core i. Output is written
as [p=(pr,o), a, c] (128 partitions) so each chunk is ONE full-width DMA;
host de-interleaves rows at the end.
"""

import math

import numpy as np

B, C, OC, SD, H = 8, 128, 64, 512, 128
KW = 3
EPS = 1e-8
HP = H + 2          # zero-padded image size in SBUF (rows/cols -1..128)
NCORES = 8
RB = 3              # image rows per chunk (PSUM bank: 3*130 = 390 <= 512 f32)
NTAP = 9            # 6 Z0 taps + 3 Z1 taps

_prog = None


def _host_z_kernels(style, weight, mod_weight, mod_bias):
    """Per-sample vertically-blur-folded kernels as matmul lhsT, f16.

    Returns (B, C, 9*128) float16 where tap t column block t*128 + pr*64 + o:
      t = d*2 + e (d row tap 0..2, e col shift 0..1)  -> Z0 taps
      t = 6 + d                                        -> Z1 taps
    """
    style = np.asarray(style, dtype=np.float64)
    weight = np.asarray(weight, dtype=np.float64)
    mod_weight = np.asarray(mod_weight, dtype=np.float64)
    mod_bias = np.asarray(mod_bias, dtype=np.float64)

    b = style.shape[0]
    scale = 1.0 / math.sqrt(C * KW * KW)
    w_gain = 1.0 / math.sqrt(mod_weight.shape[1])
    s = style @ (mod_weight.T * w_gain) + mod_bias             # (b, C)
    wmod = scale * weight * s[:, None, :, None, None]          # (b, OC, C, 3, 3)
    demod = 1.0 / np.sqrt(np.sum(wmod * wmod, axis=(2, 3, 4)) + EPS)
    wt = wmod * demod[:, :, None, None, None]                  # conv_transpose taps

    # vertical blur fold: out dilated row J = 2a+pr reads x row a+dlt with
    # FIR tap u = 2*dlt + dr + 1 - pr ; f1 = [1,3,3,1]/4, extra 0.25 for the
    # unscaled horizontal binomial stages.
    f1 = np.array([1.0, 3.0, 3.0, 1.0]) / 4.0
    V = np.zeros((2, 3, 3))
    for pr in range(2):
        for dlt in (-1, 0, 1):
            for dr in range(3):
                u = 2 * dlt + dr + 1 - pr
                if 0 <= u <= 3:
                    V[pr, dlt + 1, dr] += f1[u] * 0.25

    wts = np.zeros((b, C, NTAP * 128), dtype=np.float64)
    # wt[b, o, c, dr, dc] -> per tap block [c, pr*64+o]
    wtT = wt.transpose(0, 2, 1, 3, 4)                          # (b, C, OC, 3, 3)
    for pr in range(2):
        for d in range(3):
            # Z0: dc = 2e (e = col shift); Z1: dc = 1
            acc0 = np.zeros((b, C, OC, 2))
            acc1 = np.zeros((b, C, OC))
            for dr in range(3):
                v = V[pr, d, dr]
                if v == 0.0:
                    continue
                acc0[..., 0] += v * wtT[:, :, :, dr, 0]
                acc0[..., 1] += v * wtT[:, :, :, dr, 2]
                acc1 += v * wtT[:, :, :, dr, 1]
            for e in range(2):
                t = d * 2 + e
                wts[:, :, t * 128 + pr * 64:t * 128 + pr * 64 + OC] = acc0[..., e]
            t = 6 + d
            wts[:, :, t * 128 + pr * 64:t * 128 + pr * 64 + OC] = acc1
    return wts.astype(np.float16)


def _build():
    import concourse.bacc as bacc
    import concourse.mybir as mybir
    from concourse.tile import TileContext

    f32 = mybir.dt.float32
    f16 = mybir.dt.float16
    AOp = mybir.AluOpType

    nc = bacc.Bacc(None, target_bir_lowering=False)
    x = nc.declare_dram_parameter("x", [C, HP * HP], f16, isOutput=False)
    wts = nc.declare_dram_parameter("wts", [C, NTAP * 128], f16, isOutput=False)
    # out[p = pr*64 + o, a, c]; host interleaves rows (2a+pr) afterwards
    out = nc.declare_dram_parameter("out", [128, H, 2 * H], f32, isOutput=True)

    with TileContext(nc) as tc:
        with (
            tc.tile_pool(name="xp", bufs=1) as xpool,
            tc.tile_pool(name="wp", bufs=1) as wpool,
            tc.tile_pool(name="ps", bufs=3, space="PSUM") as pspool,
            tc.tile_pool(name="pw", bufs=1, space="PSUM") as pwpool,
            tc.tile_pool(name="zc", bufs=5) as zcpool,
            tc.tile_pool(name="ab", bufs=5) as abpool,
            tc.tile_pool(name="ob", bufs=8) as opool,
        ):
            xpad = xpool.tile([C, HP * HP], f16)
            xv = xpad[:, :].rearrange("p (r c) -> p r c", c=HP)
            wtile = wpool.tile([C, NTAP * 128], f16)

            # x arrives pre-padded; band loads so compute starts early.
            # Small first band gates only the first chunk; weights ride
            # second so the PE can start ~2.5us in.
            xdram = x.rearrange("p (r c) -> p r c", c=HP)
            nc.sync.dma_start(out=wtile[:, :], in_=wts[:, :])
            bands = [(0, 6)]
            r0 = 6
            while r0 < HP:
                r1 = min(HP, r0 + 10)
                bands.append((r0, r1))
                r0 = r1
            for r0, r1 in bands:
                nc.sync.dma_start(out=xv[:, r0:r1, :], in_=xdram[:, r0:r1, :])

            # PE p-state warmup: dummy matmuls on a zeroed tile keep the PE
            # continuously busy from ~0.5us so the ramp (3us to full clock)
            # completes before the first real chunk issues.
            warm = xpool.tile([C, 256], f16, name="warm")
            nc.gpsimd.memset(warm[:, :], 0.0)
            wps = pwpool.tile([C, 256], f32, name="warmps")
            for wi in range(13):
                nc.tensor.matmul(
                    wps[:, :], warm[:, 0:128], warm[:, :],
                    start=True, stop=True)

            sizes = [RB] * ((H - 2) // RB) + [1, 1]
            assert sum(sizes) == H
            a0 = 0
            for ci, rb in enumerate(sizes):
                z0 = pspool.tile([C, RB, H + 1], f32, tag="z0", name=f"z0_{ci}")
                z1 = pspool.tile([C, RB, H + 2], f32, tag="z1", name=f"z1_{ci}")
                z0c = zcpool.tile([C, RB, H + 1], f16, tag="z0c", name=f"z0c_{ci}")
                z1c = zcpool.tile([C, RB, H + 2], f16, tag="z1c", name=f"z1c_{ci}")
                # Z1 first: its ACT cast overlaps the 6 Z0 matmuls, shortening
                # the end-of-chunk dependency chain
                for d in range(3):
                    t = 6 + d
                    nc.tensor.matmul(
                        z1[:, 0:rb, :],
                        wtile[:, t * 128:(t + 1) * 128],
                        xv[:, a0 + d:a0 + d + rb, 0:HP],
                        start=(d == 0),
                        stop=(d == 2),
                    )
                nc.scalar.copy(z1c[:, 0:rb, :], z1[:, 0:rb, :])
                # Z0: 6 taps (d rows x e col-shifts); x col idx b-e+1
                for d in range(3):
                    for e in range(2):
                        t = d * 2 + e
                        nc.tensor.matmul(
                            z0[:, 0:rb, :],
                            wtile[:, t * 128:(t + 1) * 128],
                            xv[:, a0 + d:a0 + d + rb, 1 - e:HP - e - 1 + 1],
                            start=(t == 0),
                            stop=(t == 5),
                        )
                nc.scalar.copy(z0c[:, 0:rb, :], z0[:, 0:rb, :])
                # binomial blur stages (DVE, f16 packed 2x)
                A0 = abpool.tile([C, RB, H + 1], f16, tag="A0", name=f"A0_{ci}")
                A1 = abpool.tile([C, RB, H + 1], f16, tag="A1", name=f"A1_{ci}")
                B0 = abpool.tile([C, RB, H], f16, tag="B0", name=f"B0_{ci}")
                B1 = abpool.tile([C, RB, H + 1], f16, tag="B1", name=f"B1_{ci}")
                nc.vector.tensor_tensor(
                    out=A0[:, 0:rb, :], in0=z0c[:, 0:rb, :],
                    in1=z1c[:, 0:rb, 1:H + 2], op=AOp.add)
                nc.vector.tensor_tensor(
                    out=A1[:, 0:rb, :], in0=z1c[:, 0:rb, 0:H + 1],
                    in1=z0c[:, 0:rb, :], op=AOp.add)
                nc.vector.tensor_tensor(
                    out=B0[:, 0:rb, :], in0=A0[:, 0:rb, 0:H],
                    in1=A1[:, 0:rb, 1:H + 1], op=AOp.add)
                nc.vector.tensor_tensor(
                    out=B1[:, 0:rb, :], in0=A1[:, 0:rb, :],
                    in1=A0[:, 0:rb, :], op=AOp.add)
                # final interleaved f32 writes; Pool can't run TensorScalarPtr
                # so these are plain adds, load-balanced DVE/Pool
                osb = opool.tile([C, RB, 2 * H], f32, tag="osb", name=f"osb_{ci}")
                eng0 = nc.vector if ci % 2 == 0 else nc.gpsimd
                eng0.tensor_tensor(
                    out=osb[:, 0:rb, 0::2], in0=B1[:, 0:rb, 0:H],
                    in1=B0[:, 0:rb, :], op=AOp.add)
                nc.gpsimd.tensor_tensor(
                    out=osb[:, 0:rb, 1::2], in0=B0[:, 0:rb, :],
                    in1=B1[:, 0:rb, 1:H + 1], op=AOp.add)
                nc.sync.dma_start(
                    out=out[:, a0:a0 + rb, :], in_=osb[:, 0:rb, :])
                a0 += rb
    nc.compile()
    return nc


def _get_prog():
    global _prog
    if _prog is None:
        _prog = _build()
    return _prog


def _pad_x(xi):
    xp = np.zeros((C, HP, HP), dtype=np.float16)
    xp[:, 1:1 + H, 1:1 + H] = xi
    return xp.reshape(C, HP * HP)


def kernel(x, style, weight, mod_weight, mod_bias):
    from concourse.bass_utils import run_bass_kernel_spmd

    nc = _get_prog()
    wts = _host_z_kernels(style, weight, mod_weight, mod_bias)
    x = np.asarray(x)
    in_maps = [
        {"x": _pad_x(x[i]), "wts": np.ascontiguousarray(wts[i])}
        for i in range(NCORES)
    ]
    r = run_bass_kernel_spmd(nc, in_maps, list(range(NCORES)))
    outs = []
    for i in range(NCORES):
        o = r.results[i]["out"]                    # (128, 128, 256) [pr*64+o, a, c]
        o = o.reshape(2, OC, H, 2 * H).transpose(1, 2, 0, 3)
        outs.append(o.reshape(OC, 2 * H, 2 * H))
    return np.stack(outs, axis=0).astype(np.float32)


# revision 19
# speedup vs baseline: 1.0901x; 1.0024x over previous
"""Trainium2 Bass kernel for nn_ConvLayerWithStyleMod.

Math: reference = per-sample style-modulated 3x3 conv_transpose (stride 2)
followed by a fixed separable 4x4 blur ([1,3,3,1] outer [1,3,3,1]).

Decomposition used here (validated vs the reference to ~4e-7):
 - Fold ONLY the vertical blur axis into the conv weights. Splitting the
   dilated output grid by (row parity pr, col parity rc) leaves, per output
   row-parity, two column-parity planes Z0 (even dilated cols, 6 taps) and
   Z1 (odd dilated cols, 3 taps) -- 9 shifted matmuls per row chunk instead
   of the 18 a full 2D fold needs. Both pr phases pack into M=128.
 - The horizontal blur [1,3,3,1] = [1,1]*[1,1]*[1,1] (binomial) is three
   2-tap add stages on the cheap engines, in parity form:
     A0 = Z0[b] + Z1[b]         A1 = Z1[b] + Z0[b+1]
     B0 = A0[b] + A1[b]         B1 = A1[b] + A0[b+1]
     out0[b] = B1[b-1] + B0[b]  out1[b] = B0[b] + B1[b]
   (a global x0.25 for the two blur-axis normalizations is baked into the
   host-folded weights).
 - fp16 for x / weights / blur intermediates (PE rate is identical to f32r,
   DMA-in halves, DVE adds get the 2x packed mode); PSUM accum + final
   output stay f32.

Sharding: data-parallel over batch; sample i on core i. Output is written
as [p=(pr,o), a, c] (128 partitions) so each chunk is ONE full-width DMA;
host de-interleaves rows at the end.
"""

import math

import numpy as np

B, C, OC, SD, H = 8, 128, 64, 512, 128
KW = 3
EPS = 1e-8
HP = H + 2          # zero-padded image size in SBUF (rows/cols -1..128)
NCORES = 8
RB = 3              # image rows per chunk (PSUM bank: 3*130 = 390 <= 512 f32)
NTAP = 9            # 6 Z0 taps + 3 Z1 taps

_prog = None


def _host_z_kernels(style, weight, mod_weight, mod_bias):
    """Per-sample vertically-blur-folded kernels as matmul lhsT, f16.

    Returns (B, C, 9*128) float16 where tap t column block t*128 + pr*64 + o:
      t = d*2 + e (d row tap 0..2, e col shift 0..1)  -> Z0 taps
      t = 6 + d                                        -> Z1 taps
    """
    style = np.asarray(style, dtype=np.float64)
    weight = np.asarray(weight, dtype=np.float64)
    mod_weight = np.asarray(mod_weight, dtype=np.float64)
    mod_bias = np.asarray(mod_bias, dtype=np.float64)

    b = style.shape[0]
    scale = 1.0 / math.sqrt(C * KW * KW)
    w_gain = 1.0 / math.sqrt(mod_weight.shape[1])
    s = style @ (mod_weight.T * w_gain) + mod_bias             # (b, C)
    wmod = scale * weight * s[:, None, :, None, None]          # (b, OC, C, 3, 3)
    demod = 1.0 / np.sqrt(np.sum(wmod * wmod, axis=(2, 3, 4)) + EPS)
    wt = wmod * demod[:, :, None, None, None]                  # conv_transpose taps

    # vertical blur fold: out dilated row J = 2a+pr reads x row a+dlt with
    # FIR tap u = 2*dlt + dr + 1 - pr ; f1 = [1,3,3,1]/4, extra 0.25 for the
    # unscaled horizontal binomial stages.
    f1 = np.array([1.0, 3.0, 3.0, 1.0]) / 4.0
    V = np.zeros((2, 3, 3))
    for pr in range(2):
        for dlt in (-1, 0, 1):
            for dr in range(3):
                u = 2 * dlt + dr + 1 - pr
                if 0 <= u <= 3:
                    V[pr, dlt + 1, dr] += f1[u] * 0.25

    wts = np.zeros((b, C, NTAP * 128), dtype=np.float64)
    # wt[b, o, c, dr, dc] -> per tap block [c, pr*64+o]
    wtT = wt.transpose(0, 2, 1, 3, 4)                          # (b, C, OC, 3, 3)
    for pr in range(2):
        for d in range(3):
            # Z0: dc = 2e (e = col shift); Z1: dc = 1
            acc0 = np.zeros((b, C, OC, 2))
            acc1 = np.zeros((b, C, OC))
            for dr in range(3):
                v = V[pr, d, dr]
                if v == 0.0:
                    continue
                acc0[..., 0] += v * wtT[:, :, :, dr, 0]
                acc0[..., 1] += v * wtT[:, :, :, dr, 2]
                acc1 += v * wtT[:, :, :, dr, 1]
            for e in range(2):
                t = d * 2 + e
                wts[:, :, t * 128 + pr * 64:t * 128 + pr * 64 + OC] = acc0[..., e]
            t = 6 + d
            wts[:, :, t * 128 + pr * 64:t * 128 + pr * 64 + OC] = acc1
    return wts.astype(np.float16)


def _build():
    import concourse.bacc as bacc
    import concourse.mybir as mybir
    from concourse.tile import TileContext

    f32 = mybir.dt.float32
    f16 = mybir.dt.float16
    AOp = mybir.AluOpType

    nc = bacc.Bacc(None, target_bir_lowering=False)
    x = nc.declare_dram_parameter("x", [C, HP * HP], f16, isOutput=False)
    wts = nc.declare_dram_parameter("wts", [C, NTAP * 128], f16, isOutput=False)
    # out[p = pr*64 + o, a, c]; host interleaves rows (2a+pr) afterwards
    out = nc.declare_dram_parameter("out", [128, H, 2 * H], f32, isOutput=True)

    with TileContext(nc) as tc:
        with (
            tc.tile_pool(name="xp", bufs=1) as xpool,
            tc.tile_pool(name="wp", bufs=1) as wpool,
            tc.tile_pool(name="ps", bufs=3, space="PSUM") as pspool,
            tc.tile_pool(name="pw", bufs=1, space="PSUM") as pwpool,
            tc.tile_pool(name="zc", bufs=3) as zcpool,
            tc.tile_pool(name="ab", bufs=3) as abpool,
            tc.tile_pool(name="ob", bufs=4) as opool,
        ):
            xpad = xpool.tile([C, HP * HP], f16)
            xv = xpad[:, :].rearrange("p (r c) -> p r c", c=HP)
            wtile = wpool.tile([C, NTAP * 128], f16)

            # x arrives pre-padded; band loads so compute starts early.
            # Small first band gates only the first chunk; weights ride
            # second so the PE can start ~2.5us in.
            xdram = x.rearrange("p (r c) -> p r c", c=HP)
            nc.sync.dma_start(out=wtile[:, :], in_=wts[:, :])
            bands = [(0, 5)]
            r0 = 5
            while r0 < HP:
                r1 = min(HP, r0 + 10)
                bands.append((r0, r1))
                r0 = r1
            for r0, r1 in bands:
                nc.sync.dma_start(out=xv[:, r0:r1, :], in_=xdram[:, r0:r1, :])

            # PE p-state warmup: dummy matmuls on a zeroed tile keep the PE
            # continuously busy from ~0.5us so the ramp (3us to full clock)
            # completes before the first real chunk issues.
            warm = xpool.tile([C, 256], f16, name="warm")
            nc.gpsimd.memset(warm[:, :], 0.0)
            wps = pwpool.tile([C, 256], f32, name="warmps")
            for wi in range(11):
                nc.tensor.matmul(
                    wps[:, :], warm[:, 0:128], warm[:, :],
                    start=True, stop=True)

            # (pair of sub-chunk row counts); the last pair is tiny so the
            # end-of-kernel drain chain is short
            pairs = [(RB, RB)] * 20 + [(2, 2), (1, 1), (1, 1)]
            assert sum(r for p in pairs for r in p) == H
            a0 = 0
            for pi, (rba, rbb) in enumerate(pairs):
                rb = rba
                # shared 2-chunk cast tiles: the blur ladder runs once per
                # pair at double width, halving DVE/Pool op count+overhead
                z0c = zcpool.tile([C, 2, RB, H + 1], f16, tag="z0c", name=f"z0c_{pi}")
                z1c = zcpool.tile([C, 2, RB, H + 2], f16, tag="z1c", name=f"z1c_{pi}")
                for j in range(2):
                    aj = a0 + j * rb
                    z0 = pspool.tile([C, RB, H + 1], f32, tag="z0", name=f"z0_{pi}_{j}")
                    z1 = pspool.tile([C, RB, H + 2], f32, tag="z1", name=f"z1_{pi}_{j}")
                    # Z1 first: its ACT cast overlaps the 6 Z0 matmuls
                    for d in range(3):
                        t = 6 + d
                        nc.tensor.matmul(
                            z1[:, 0:rb, :],
                            wtile[:, t * 128:(t + 1) * 128],
                            xv[:, aj + d:aj + d + rb, 0:HP],
                            start=(d == 0),
                            stop=(d == 2),
                        )
                    nc.scalar.copy(z1c[:, j, 0:rb, :], z1[:, 0:rb, :])
                    # Z0: 6 taps (d rows x e col-shifts); x col idx b-e+1
                    for d in range(3):
                        for e in range(2):
                            t = d * 2 + e
                            nc.tensor.matmul(
                                z0[:, 0:rb, :],
                                wtile[:, t * 128:(t + 1) * 128],
                                xv[:, aj + d:aj + d + rb, 1 - e:HP - e - 1 + 1],
                                start=(t == 0),
                                stop=(t == 5),
                            )
                    nc.scalar.copy(z0c[:, j, 0:rb, :], z0[:, 0:rb, :])
                # binomial blur stages (DVE, f16 packed 2x), pair-batched
                A0 = abpool.tile([C, 2, RB, H + 1], f16, tag="A0", name=f"A0_{pi}")
                A1 = abpool.tile([C, 2, RB, H + 1], f16, tag="A1", name=f"A1_{pi}")
                B0 = abpool.tile([C, 2, RB, H], f16, tag="B0", name=f"B0_{pi}")
                B1 = abpool.tile([C, 2, RB, H + 1], f16, tag="B1", name=f"B1_{pi}")
                nc.vector.tensor_tensor(
                    out=A0[:, :, 0:rb, :], in0=z0c[:, :, 0:rb, :],
                    in1=z1c[:, :, 0:rb, 1:H + 2], op=AOp.add)
                nc.vector.tensor_tensor(
                    out=A1[:, :, 0:rb, :], in0=z1c[:, :, 0:rb, 0:H + 1],
                    in1=z0c[:, :, 0:rb, :], op=AOp.add)
                nc.vector.tensor_tensor(
                    out=B0[:, :, 0:rb, :], in0=A0[:, :, 0:rb, 0:H],
                    in1=A1[:, :, 0:rb, 1:H + 1], op=AOp.add)
                nc.vector.tensor_tensor(
                    out=B1[:, :, 0:rb, :], in0=A1[:, :, 0:rb, :],
                    in1=A0[:, :, 0:rb, :], op=AOp.add)
                # final interleaved f32 writes, load-balanced DVE/Pool
                osb = opool.tile([C, 2, RB, 2 * H], f32, tag="osb", name=f"osb_{pi}")
                eng0 = nc.vector if pi % 2 == 0 else nc.gpsimd
                eng0.tensor_tensor(
                    out=osb[:, :, 0:rb, 0::2], in0=B1[:, :, 0:rb, 0:H],
                    in1=B0[:, :, 0:rb, :], op=AOp.add)
                nc.gpsimd.tensor_tensor(
                    out=osb[:, :, 0:rb, 1::2], in0=B0[:, :, 0:rb, :],
                    in1=B1[:, :, 0:rb, 1:H + 1], op=AOp.add)
                dest = out[:, a0:a0 + 2 * rb, :].rearrange(
                    "p (two r) c -> p two r c", r=rb)
                nc.sync.dma_start(out=dest, in_=osb[:, :, 0:rb, :])
                a0 += 2 * rb
    nc.compile()
    return nc


def _get_prog():
    global _prog
    if _prog is None:
        _prog = _build()
    return _prog


def _pad_x(xi):
    xp = np.zeros((C, HP, HP), dtype=np.float16)
    xp[:, 1:1 + H, 1:1 + H] = xi
    return xp.reshape(C, HP * HP)


def kernel(x, style, weight, mod_weight, mod_bias):
    from concourse.bass_utils import run_bass_kernel_spmd

    nc = _get_prog()
    wts = _host_z_kernels(style, weight, mod_weight, mod_bias)
    x = np.asarray(x)
    in_maps = [
        {"x": _pad_x(x[i]), "wts": np.ascontiguousarray(wts[i])}
        for i in range(NCORES)
    ]
    r = run_bass_kernel_spmd(nc, in_maps, list(range(NCORES)))
    outs = []
    for i in range(NCORES):
        o = r.results[i]["out"]                    # (128, 128, 256) [pr*64+o, a, c]
        o = o.reshape(2, OC, H, 2 * H).transpose(1, 2, 0, 3)
        outs.append(o.reshape(OC, 2 * H, 2 * H))
    return np.stack(outs, axis=0).astype(np.float32)


# revision 23
# speedup vs baseline: 1.0986x; 1.0078x over previous
"""Trainium2 Bass kernel for nn_ConvLayerWithStyleMod.

Math: reference = per-sample style-modulated 3x3 conv_transpose (stride 2)
followed by a fixed separable 4x4 blur ([1,3,3,1] outer [1,3,3,1]).

Decomposition used here (validated vs the reference to ~4e-7):
 - Fold ONLY the vertical blur axis into the conv weights. Splitting the
   dilated output grid by (row parity pr, col parity rc) leaves, per output
   row-parity, two column-parity planes Z0 (even dilated cols, 6 taps) and
   Z1 (odd dilated cols, 3 taps) -- 9 shifted matmuls per row chunk instead
   of the 18 a full 2D fold needs. Both pr phases pack into M=128.
 - The horizontal blur [1,3,3,1] = [1,1]*[1,1]*[1,1] (binomial) is three
   2-tap add stages on the cheap engines, in parity form:
     A0 = Z0[b] + Z1[b]         A1 = Z1[b] + Z0[b+1]
     B0 = A0[b] + A1[b]         B1 = A1[b] + A0[b+1]
     out0[b] = B1[b-1] + B0[b]  out1[b] = B0[b] + B1[b]
   (a global x0.25 for the two blur-axis normalizations is baked into the
   host-folded weights).
 - fp16 for x / weights / blur intermediates (PE rate is identical to f32r,
   DMA-in halves, DVE adds get the 2x packed mode); PSUM accum + final
   output stay f32.

Sharding: data-parallel over batch; sample i on core i. Output is written
as [p=(pr,o), a, c] (128 partitions) so each chunk is ONE full-width DMA;
host de-interleaves rows at the end.
"""

import math

import numpy as np

B, C, OC, SD, H = 8, 128, 64, 512, 128
KW = 3
EPS = 1e-8
HP = H + 2          # zero-padded image size in SBUF (rows/cols -1..128)
NCORES = 8
RB = 3              # image rows per chunk (PSUM bank: 3*130 = 390 <= 512 f32)
NTAP = 9            # 6 Z0 taps + 3 Z1 taps

_prog = None


def _host_z_kernels(style, weight, mod_weight, mod_bias):
    """Per-sample vertically-blur-folded kernels as matmul lhsT, f16.

    Returns (B, C, 9*128) float16 where tap t column block t*128 + pr*64 + o:
      t = d*2 + e (d row tap 0..2, e col shift 0..1)  -> Z0 taps
      t = 6 + d                                        -> Z1 taps
    """
    style = np.asarray(style, dtype=np.float64)
    weight = np.asarray(weight, dtype=np.float64)
    mod_weight = np.asarray(mod_weight, dtype=np.float64)
    mod_bias = np.asarray(mod_bias, dtype=np.float64)

    b = style.shape[0]
    scale = 1.0 / math.sqrt(C * KW * KW)
    w_gain = 1.0 / math.sqrt(mod_weight.shape[1])
    s = style @ (mod_weight.T * w_gain) + mod_bias             # (b, C)
    wmod = scale * weight * s[:, None, :, None, None]          # (b, OC, C, 3, 3)
    demod = 1.0 / np.sqrt(np.sum(wmod * wmod, axis=(2, 3, 4)) + EPS)
    wt = wmod * demod[:, :, None, None, None]                  # conv_transpose taps

    # vertical blur fold: out dilated row J = 2a+pr reads x row a+dlt with
    # FIR tap u = 2*dlt + dr + 1 - pr ; f1 = [1,3,3,1]/4, extra 0.25 for the
    # unscaled horizontal binomial stages.
    f1 = np.array([1.0, 3.0, 3.0, 1.0]) / 4.0
    V = np.zeros((2, 3, 3))
    for pr in range(2):
        for dlt in (-1, 0, 1):
            for dr in range(3):
                u = 2 * dlt + dr + 1 - pr
                if 0 <= u <= 3:
                    V[pr, dlt + 1, dr] += f1[u] * 0.25

    wts = np.zeros((b, C, NTAP * 128), dtype=np.float64)
    # wt[b, o, c, dr, dc] -> per tap block [c, pr*64+o]
    wtT = wt.transpose(0, 2, 1, 3, 4)                          # (b, C, OC, 3, 3)
    for pr in range(2):
        for d in range(3):
            # Z0: dc = 2e (e = col shift); Z1: dc = 1
            acc0 = np.zeros((b, C, OC, 2))
            acc1 = np.zeros((b, C, OC))
            for dr in range(3):
                v = V[pr, d, dr]
                if v == 0.0:
                    continue
                acc0[..., 0] += v * wtT[:, :, :, dr, 0]
                acc0[..., 1] += v * wtT[:, :, :, dr, 2]
                acc1 += v * wtT[:, :, :, dr, 1]
            for e in range(2):
                t = d * 2 + e
                wts[:, :, t * 128 + pr * 64:t * 128 + pr * 64 + OC] = acc0[..., e]
            t = 6 + d
            wts[:, :, t * 128 + pr * 64:t * 128 + pr * 64 + OC] = acc1
    return wts.astype(np.float16)


def _build():
    import concourse.bacc as bacc
    import concourse.mybir as mybir
    from concourse.tile import TileContext

    f32 = mybir.dt.float32
    f16 = mybir.dt.float16
    AOp = mybir.AluOpType

    nc = bacc.Bacc(None, target_bir_lowering=False)
    x = nc.declare_dram_parameter("x", [C, HP * HP], f16, isOutput=False)
    wts = nc.declare_dram_parameter("wts", [C, NTAP * 128], f16, isOutput=False)
    # out[p = pr*64 + o, a, c]; host interleaves rows (2a+pr) afterwards
    out = nc.declare_dram_parameter("out", [128, H, 2 * H], f32, isOutput=True)

    with TileContext(nc) as tc:
        with (
            tc.tile_pool(name="xp", bufs=1) as xpool,
            tc.tile_pool(name="wp", bufs=1) as wpool,
            tc.tile_pool(name="ps", bufs=3, space="PSUM") as pspool,
            tc.tile_pool(name="pw", bufs=1, space="PSUM") as pwpool,
            tc.tile_pool(name="zc", bufs=3) as zcpool,
            tc.tile_pool(name="ab", bufs=3) as abpool,
            tc.tile_pool(name="ob", bufs=4) as opool,
        ):
            xpad = xpool.tile([C, HP * HP], f16)
            xv = xpad[:, :].rearrange("p (r c) -> p r c", c=HP)
            wtile = wpool.tile([C, NTAP * 128], f16)

            # x arrives pre-padded; band loads so compute starts early.
            # Small first band gates only the first chunk; weights ride
            # second so the PE can start ~2.5us in.
            xdram = x.rearrange("p (r c) -> p r c", c=HP)
            nc.sync.dma_start(out=wtile[:, :], in_=wts[:, :])
            bands = [(0, 5)]
            r0 = 5
            while r0 < HP:
                r1 = min(HP, r0 + 10)
                bands.append((r0, r1))
                r0 = r1
            for r0, r1 in bands:
                nc.sync.dma_start(out=xv[:, r0:r1, :], in_=xdram[:, r0:r1, :])

            # PE p-state warmup: dummy matmuls on a zeroed tile keep the PE
            # continuously busy from ~0.5us so the ramp (3us to full clock)
            # completes before the first real chunk issues.
            warm = xpool.tile([C, 256], f16, name="warm")
            nc.gpsimd.memset(warm[:, :], 0.0)
            wps = pwpool.tile([C, 256], f32, name="warmps")
            for wi in range(11):
                nc.tensor.matmul(
                    wps[:, :], warm[:, 0:128], warm[:, :],
                    start=True, stop=True)

            # (n_subchunks, rows per subchunk); big pair-batched groups in
            # steady state, telescoping singles at the end so the final
            # drain chain (casts -> ladder -> finals -> DMA) is short
            groups = [(2, RB)] * 20 + [(2, 2), (1, 2), (1, 2)]
            assert sum(ns * r for ns, r in groups) == H
            ngroups = len(groups)
            a0 = 0
            for pi, (ns, rb) in enumerate(groups):
                # shared 2-chunk cast tiles: the blur ladder runs once per
                # pair at double width, halving DVE/Pool op count+overhead
                z0c = zcpool.tile([C, 2, RB, H + 1], f16, tag="z0c", name=f"z0c_{pi}")
                z1c = zcpool.tile([C, 2, RB, H + 2], f16, tag="z1c", name=f"z1c_{pi}")
                for j in range(ns):
                    aj = a0 + j * rb
                    z0 = pspool.tile([C, RB, H + 1], f32, tag="z0", name=f"z0_{pi}_{j}")
                    z1 = pspool.tile([C, RB, H + 2], f32, tag="z1", name=f"z1_{pi}_{j}")
                    # Z1 first: its ACT cast overlaps the 6 Z0 matmuls
                    for d in range(3):
                        t = 6 + d
                        nc.tensor.matmul(
                            z1[:, 0:rb, :],
                            wtile[:, t * 128:(t + 1) * 128],
                            xv[:, aj + d:aj + d + rb, 0:HP],
                            start=(d == 0),
                            stop=(d == 2),
                        )
                    nc.scalar.copy(z1c[:, j, 0:rb, :], z1[:, 0:rb, :])
                    # Z0: 6 taps (d rows x e col-shifts); x col idx b-e+1
                    for d in range(3):
                        for e in range(2):
                            t = d * 2 + e
                            nc.tensor.matmul(
                                z0[:, 0:rb, :],
                                wtile[:, t * 128:(t + 1) * 128],
                                xv[:, aj + d:aj + d + rb, 1 - e:HP - e - 1 + 1],
                                start=(t == 0),
                                stop=(t == 5),
                            )
                    nc.scalar.copy(z0c[:, j, 0:rb, :], z0[:, 0:rb, :])
                # binomial blur stages (DVE, f16 packed 2x), pair-batched
                A0 = abpool.tile([C, 2, RB, H + 1], f16, tag="A0", name=f"A0_{pi}")
                A1 = abpool.tile([C, 2, RB, H + 1], f16, tag="A1", name=f"A1_{pi}")
                B0 = abpool.tile([C, 2, RB, H], f16, tag="B0", name=f"B0_{pi}")
                B1 = abpool.tile([C, 2, RB, H + 1], f16, tag="B1", name=f"B1_{pi}")
                nc.vector.tensor_tensor(
                    out=A0[:, 0:ns, 0:rb, :], in0=z0c[:, 0:ns, 0:rb, :],
                    in1=z1c[:, 0:ns, 0:rb, 1:H + 2], op=AOp.add)
                nc.vector.tensor_tensor(
                    out=A1[:, 0:ns, 0:rb, :], in0=z1c[:, 0:ns, 0:rb, 0:H + 1],
                    in1=z0c[:, 0:ns, 0:rb, :], op=AOp.add)
                nc.vector.tensor_tensor(
                    out=B0[:, 0:ns, 0:rb, :], in0=A0[:, 0:ns, 0:rb, 0:H],
                    in1=A1[:, 0:ns, 0:rb, 1:H + 1], op=AOp.add)
                nc.vector.tensor_tensor(
                    out=B1[:, 0:ns, 0:rb, :], in0=A1[:, 0:ns, 0:rb, :],
                    in1=A0[:, 0:ns, 0:rb, :], op=AOp.add)
                # final interleaved f32 writes, split per subchunk and
                # spread across DVE+Pool so each subchunk's output DMA can
                # ship as soon as its two finals land (shorter drain chain)
                osb = opool.tile([C, 2, RB, 2 * H], f32, tag="osb", name=f"osb_{pi}")
                for j in range(ns):
                    engs = (nc.vector, nc.gpsimd) if j == 0 else (nc.gpsimd, nc.vector)
                    engs[0].tensor_tensor(
                        out=osb[:, j, 0:rb, 0::2], in0=B1[:, j, 0:rb, 0:H],
                        in1=B0[:, j, 0:rb, :], op=AOp.add)
                    engs[1].tensor_tensor(
                        out=osb[:, j, 0:rb, 1::2], in0=B0[:, j, 0:rb, :],
                        in1=B1[:, j, 0:rb, 1:H + 1], op=AOp.add)
                    nc.sync.dma_start(
                        out=out[:, a0 + j * rb:a0 + (j + 1) * rb, :],
                        in_=osb[:, j, 0:rb, :])
                a0 += ns * rb
    nc.compile()
    return nc


def _get_prog():
    global _prog
    if _prog is None:
        _prog = _build()
    return _prog


def _pad_x(xi):
    xp = np.zeros((C, HP, HP), dtype=np.float16)
    xp[:, 1:1 + H, 1:1 + H] = xi
    return xp.reshape(C, HP * HP)


def kernel(x, style, weight, mod_weight, mod_bias):
    from concourse.bass_utils import run_bass_kernel_spmd

    nc = _get_prog()
    wts = _host_z_kernels(style, weight, mod_weight, mod_bias)
    x = np.asarray(x)
    in_maps = [
        {"x": _pad_x(x[i]), "wts": np.ascontiguousarray(wts[i])}
        for i in range(NCORES)
    ]
    r = run_bass_kernel_spmd(nc, in_maps, list(range(NCORES)))
    outs = []
    for i in range(NCORES):
        o = r.results[i]["out"]                    # (128, 128, 256) [pr*64+o, a, c]
        o = o.reshape(2, OC, H, 2 * H).transpose(1, 2, 0, 3)
        outs.append(o.reshape(OC, 2 * H, 2 * H))
    return np.stack(outs, axis=0).astype(np.float32)
